# revision 14
# baseline (speedup 1.0000x reference)
"""Trainium2 Bass kernel for nn_ConvShare: multi-width causal conv + shared projection.

Reference computation (per batch element b):
    xpad = pad(x[b], L -> L+W-1)                       # [L+11, D]
    taps[k]  = xpad[k:k+L, :] @ conv_w[:, :, k].T      # [L, D], k = 0..W-1
    spans[k] = cumsum_k taps                           # [L, D]
    h[k]     = relu(spans[k])
    out[:, k, :] = h[k] @ proj_w.T + proj_b            # [L, W, D]

Sharding: data-parallel over batch B=8 across the 8 NeuronCores (no
communication; conv_w/proj_w replicated per core).

On-chip layout is feature-major ([D, L], contraction dim on SBUF
partitions) for the conv stage; the proj stage streams h as the moving
operand so its output lands feature-major [o, L] and DMAs to a [W, D, L]
DRAM buffer (host transposes to [L, W, D]).

V2 structure (vs the 211us V1 baseline):
  - conv cumsum accumulates directly in 6 persistent PSUM banks
    (CUMSUM="psum"): no vector-engine adds at all.
  - proj bias+evacuation on the scalar engine (Identity activation with
    per-partition bias), so the vector engine is idle: scalar does
    6 relu + 6 evac per tap (~5us) under the PE's 15us/tap.
  - fp16 output (halves the output DMA; host upcasts to fp32).
  - WARMUP dummy matmuls with no data deps keep the PE busy from the
    startup barrier so the HAM clock gate (1.2 -> 2.4 GHz after ~3.4us
    of sustained busy) opens before the first real matmul instead of
    4us into the real stream.
  - conv tap k only streams N=512-k columns (the zero-pad tail would
    add zero; PSUM keeps the previous cumsum there), so xT needs no pad.
  - cw prefetch depth 2 and pw loaded after cw[1]: the early-tap DMA
    supply is marginal at warm clock, this removes the tap-1/2 stalls.

MODE selects the matmul input dtype (PSUM accumulation is fp32):
  "f16" (default) / "bf16" / "f32r".
"""

import os
import sys

import numpy as np

if True:  # make concourse importable regardless of harness cwd
    for _p in ("/opt/trn_rl_repo", "/opt/pypackages"):
        if _p not in sys.path and os.path.isdir(_p):
            sys.path.append(_p)

from contextlib import ExitStack  # noqa: E402

import ml_dtypes  # noqa: E402

import concourse.bacc as bacc  # noqa: E402
import concourse.bass as bass  # noqa: E402
import concourse.mybir as mybir  # noqa: E402
import concourse.tile as tile  # noqa: E402
from concourse import bass_utils  # noqa: E402

B, L, D, W = 8, 512, 768, 12
P = 128          # SBUF partitions
C = D // P       # 6 contraction chunks of 128
LP = L + W - 1   # 523: right-padded sequence length (legacy layouts only)
NB = L // P      # 4 output row blocks for proj

F32 = mybir.dt.float32
F16 = mybir.dt.float16
RELU = mybir.ActivationFunctionType.Relu
IDENT = mybir.ActivationFunctionType.Identity

MODE = "f16"     # "f32r" | "bf16" | "f16"
CUMSUM = "psum"  # "sbuf" | "psum"
STRUCT = "fmajor"  # "fmajor" ([o2,l] out, host transpose) | "lmajor" ([l,o2], direct DMA)
OUT16 = True     # fp16 DRAM output, host upcast (halves output DMA)
WARMUP = 7       # dummy N=512 matmuls. The PE clears its preamble at ~7.0us
                 # but the first input chunk lands at 10.0-11.6us (DMA ring
                 # spin-up + ~245 GB/s early rate); 7 dummies cover that dead
                 # window (each 427ns at the cold 1.2 GHz clock) and open the
                 # HAM clock gate (~3.4us sustained busy) so the real stream
                 # runs warm from its first matmul (~0.8us measured win).
TRIM = True      # conv tap k streams only N=512-k columns
PREFETCH = 2     # cw tap prefetch depth
DELAY_PROJ = True  # issue proj(k) after conv(k+1): +7.7us of DMA lead for cw
EVAC = "vector"  # proj psum evacuation engine: "vector" | "scalar"

# Knobs the test harness may flip before calling kernel():
TRACE = False
LAST_RESULTS = None


def _build_program(mode: str, cumsum: str, struct: str) -> bass.Bass:
    mdt = {
        "f32r": mybir.dt.float32r,
        "bf16": mybir.dt.bfloat16,
        "f16": mybir.dt.float16,
    }[mode]
    odt = F16 if OUT16 else F32

    nc = bacc.Bacc(
        "TRN2",
        target_bir_lowering=False,
        debug=False,
        num_devices=B,
    )

    # DRAM I/O. Matmul inputs are pre-chunked host-side to [C, P, n] so each
    # chunk DMA is a clean 2D copy and compute can start on chunk 0 early.
    xlen = L if TRIM else LP
    xT = nc.dram_tensor("xT", [C, P, xlen], mdt, kind="ExternalInput").ap()
    cw = nc.dram_tensor("cw", [W, C, P, D], mdt, kind="ExternalInput").ap()
    pw = nc.dram_tensor("pw", [C, P, D], mdt, kind="ExternalInput").ap()
    if struct == "fmajor":
        pb = nc.dram_tensor("pb", [P, C], F32, kind="ExternalInput").ap()
        out = nc.dram_tensor("out", [W, D, L], odt, kind="ExternalOutput").ap()
    else:
        pb = nc.dram_tensor("pb", [P, D], F32, kind="ExternalInput").ap()
        out = nc.dram_tensor("out", [L, W, D], odt, kind="ExternalOutput").ap()

    with tile.TileContext(nc) as tc, ExitStack() as ctx:
        const_pool = ctx.enter_context(tc.tile_pool(name="const", bufs=1))
        cw_pool = ctx.enter_context(tc.tile_pool(name="cw", bufs=1 + PREFETCH))
        h_pool = ctx.enter_context(tc.tile_pool(name="h", bufs=2))
        out_pool = ctx.enter_context(tc.tile_pool(name="out", bufs=4))
        if cumsum == "psum":
            psc_pool = ctx.enter_context(tc.tile_pool(name="psc", bufs=1, space="PSUM"))
            psp_pool = ctx.enter_context(tc.tile_pool(name="psp", bufs=2, space="PSUM"))
        else:
            psc_pool = ctx.enter_context(tc.tile_pool(name="psc", bufs=4, space="PSUM"))
            psp_pool = ctx.enter_context(tc.tile_pool(name="psp", bufs=4, space="PSUM"))

        if WARMUP:
            # Dummy matmuls with no DMA dependencies: they run from the
            # moment the PE clears the startup barrier (~6us) and hold it
            # busy until the first input chunks land (~9.5us), so the HAM
            # clock gate (needs ~3.4us sustained busy) opens right as the
            # real stream begins. Same dtype/shape class as the real
            # matmuls (fp32 dummies hang the HW).
            wa = const_pool.tile([P, P], mdt, name="warm_a")
            wb = const_pool.tile([P, 512], mdt, name="warm_b")
            nc.gpsimd.memset(wa[:], 0.0)
            nc.gpsimd.memset(wb[:], 0.0)
            for wi in range(WARMUP):
                wp = psp_pool.tile([P, 512], F32, tag="psp", name=f"warm_ps{wi}")
                nc.tensor.matmul(
                    wp[:], lhsT=wa[:], rhs=wb[:], start=True, stop=True
                )

        def load_cw(k):
            ts = []
            for c in range(C):
                t = cw_pool.tile([P, D], mdt, tag=f"cw{c}", name=f"cw{c}_{k}")
                nc.sync.dma_start(t[:], cw[k, c, :, :])
                ts.append(t)
            return ts

        # Interleave the startup loads so the first conv matmuls (which need
        # cw[0] chunk c + xT chunk c) can begin as soon as chunk 0 lands.
        cw_tiles = {}
        xT_t = []
        for c in range(C):
            t = cw_pool.tile([P, D], mdt, tag=f"cw{c}", name=f"cw{c}_0")
            nc.sync.dma_start(t[:], cw[0, c, :, :])
            cw_tiles.setdefault(0, []).append(t)
            xt = const_pool.tile([P, xlen], mdt, tag=f"xt{c}", name=f"xt{c}")
            nc.sync.dma_start(xt[:], xT[c, :, :])
            xT_t.append(xt)

        # cw[1] interleaved chunk-wise with pw: the early phase is
        # DMA-supply bound (~245 GB/s), and conv(1) consumes cw[1] chunks
        # progressively while proj(0) starts right after conv(1) —
        # interleaving satisfies both demand curves with ~no stall
        # (cw1-then-pw stalls proj(0) by ~1us; pw-then-cw1 stalls conv(1)
        # by much more). pb LAST: tiny-row DMAs take the serial
        # DMA_DIRECT2D path on Sync (~650ns each) and must not delay the
        # descriptor writes of the bulk loads.
        pw_t = []
        for c in range(C):
            if 1 < min(PREFETCH, W):
                t = cw_pool.tile([P, D], mdt, tag=f"cw{c}", name=f"cw{c}_1")
                nc.sync.dma_start(t[:], cw[1, c, :, :])
                cw_tiles.setdefault(1, []).append(t)
            t = const_pool.tile([P, D], mdt, tag=f"pw{c}", name=f"pw{c}")
            nc.sync.dma_start(t[:], pw[c, :, :])
            pw_t.append(t)
        for k in range(2, min(PREFETCH, W)):
            cw_tiles[k] = load_cw(k)

        if struct == "fmajor":
            # single [P, C] tile: pb_tile[p, o2b] = proj_b[o2b*P + p]
            pb_tile = const_pool.tile([P, C], F32, name="pb")
            nc.sync.dma_start(pb_tile[:], pb[:, :])
            pb_t = [pb_tile[:, o2b : o2b + 1] for o2b in range(C)]
        else:
            pb_t = const_pool.tile([P, D], F32)
            nc.sync.dma_start(pb_t[:], pb[:])

        if cumsum == "psum":
            # 6 persistent PSUM banks accumulate the conv cumsum across taps.
            sp_acc = [
                psc_pool.tile([P, L], F32, tag=f"sp{ob}", name=f"sp{ob}")
                for ob in range(C)
            ]
            spans = None
        else:
            spans = const_pool.tile([P, C * L], F32)      # running conv cumsum
            nc.gpsimd.memset(spans[:], 0.0)

        def conv_stage(k, cw_cur):
            # --- conv tap k: psum[o_blk, l] += sum_d cw^T[d, o] * x^T[d, l+k]
            nk = L - k if TRIM else L
            h_t = [h_pool.tile([P, L], mdt, tag=f"h{c}", name=f"h{c}_{k}") for c in range(C)]
            for ob in range(C):
                if cumsum == "psum":
                    ps = sp_acc[ob]
                    for c in range(C):
                        nc.tensor.matmul(
                            ps[:, 0:nk],
                            lhsT=cw_cur[c][:, ob * P : (ob + 1) * P],
                            rhs=xT_t[c][:, k : k + nk],
                            start=(k == 0 and c == 0),
                            stop=(k == W - 1 and c == C - 1),
                            skip_group_check=True,
                        )
                    nc.scalar.activation(h_t[ob][:], ps[:], RELU)
                else:
                    ps = psc_pool.tile([P, L], F32, tag="psc")
                    for c in range(C):
                        nc.tensor.matmul(
                            ps[:, 0:nk],
                            lhsT=cw_cur[c][:, ob * P : (ob + 1) * P],
                            rhs=xT_t[c][:, k : k + nk],
                            start=(c == 0),
                            stop=(c == C - 1),
                        )
                    sp = spans[:, ob * L : (ob + 1) * L]
                    nc.vector.tensor_add(sp[0:P, 0:nk], sp[0:P, 0:nk], ps[:, 0:nk])
                    nc.scalar.activation(h_t[ob][:], sp, RELU)
            return h_t

        def proj_stage(k, h_t):
            if struct == "fmajor":
                # --- proj tap k (feature-major): out^T[o2_blk, l] =
                #     sum_d pw^T[d, o2] * h^T[d, l]; 36 N=512 matmuls.
                for o2b in range(C):
                    o_t = out_pool.tile([P, L], odt, tag="out", name=f"o_{k}_{o2b}")
                    pp = psp_pool.tile([P, 512], F32, tag="psp", name=f"pp_{k}_{o2b}")
                    for c in range(C):
                        nc.tensor.matmul(
                            pp[:],
                            lhsT=pw_t[c][:, o2b * P : (o2b + 1) * P],
                            rhs=h_t[c][:],
                            start=(c == 0),
                            stop=(c == C - 1),
                        )
                    # NOTE: do NOT split this into smaller column chunks —
                    # rows below ~512B flip the DMA onto the serial
                    # DMA_DIRECT2D path on the Sync engine (~650ns each),
                    # which is slower than one hardware-queue DMA.
                    if EVAC == "scalar":
                        nc.scalar.activation(o_t[:], pp[:], IDENT, bias=pb_t[o2b])
                    else:
                        nc.vector.tensor_scalar_add(o_t[:], pp[:], pb_t[o2b])
                    nc.sync.dma_start(out[k, o2b * P : (o2b + 1) * P, :], o_t[:])
            else:
                # --- proj tap k: out[l_blk, o2] = sum_d h^T[d, l]*pw^T[d, o2]+b
                for lb in range(NB):
                    o_t = out_pool.tile([P, D], odt, tag="out")
                    for n0, nn in ((0, 512), (512, 256)):
                        pp = psp_pool.tile([P, 512], F32, tag="psp")
                        for c in range(C):
                            nc.tensor.matmul(
                                pp[:, 0:nn],
                                lhsT=h_t[c][:, lb * P : (lb + 1) * P],
                                rhs=pw_t[c][:, n0 : n0 + nn],
                                start=(c == 0),
                                stop=(c == C - 1),
                            )
                        nc.vector.tensor_add(
                            o_t[:, n0 : n0 + nn], pp[:, 0:nn], pb_t[:, n0 : n0 + nn]
                        )
                    nc.sync.dma_start(out[lb * P : (lb + 1) * P, k, :], o_t[:])

        # Pipeline. With DELAY_PROJ the PE order is conv(0), conv(1),
        # proj(0), conv(2), proj(1), ..., conv(11), proj(10), proj(11):
        # each tap's cw DMA gets a full extra proj stage (~7.7us) of lead
        # time, which removes the tap-1/2 input stalls at warm clock.
        h_prev = None
        for k in range(W):
            if k + PREFETCH < W:
                cw_tiles[k + PREFETCH] = load_cw(k + PREFETCH)
            h_t = conv_stage(k, cw_tiles.pop(k))
            if not DELAY_PROJ:
                proj_stage(k, h_t)
            else:
                if h_prev is not None:
                    proj_stage(k - 1, h_prev)
                h_prev = h_t
        if DELAY_PROJ:
            proj_stage(W - 1, h_prev)

    nc.compile()
    return nc


_program_cache: dict = {}


def _get_program(mode: str, cumsum: str = None, struct: str = None) -> bass.Bass:
    if cumsum is None:
        cumsum = CUMSUM
    if struct is None:
        struct = STRUCT
    key = (mode, cumsum, struct, WARMUP, OUT16, TRIM, PREFETCH, DELAY_PROJ, EVAC)
    if key not in _program_cache:
        _program_cache[key] = _build_program(mode, cumsum, struct)
    return _program_cache[key]


def _np_dt(mode: str):
    return {"f32r": np.float32, "bf16": ml_dtypes.bfloat16, "f16": np.float16}[mode]


def _prep_inputs(x, conv_w, proj_w, proj_b, mode: str):
    x = np.asarray(x, dtype=np.float32)
    conv_w = np.asarray(conv_w, dtype=np.float32)
    proj_w = np.asarray(proj_w, dtype=np.float32)
    proj_b = np.asarray(proj_b, dtype=np.float32)
    ndt = _np_dt(mode)

    if TRIM:
        xT_all = np.ascontiguousarray(
            x.transpose(0, 2, 1).reshape(B, C, P, L).astype(ndt)
        )                                                        # [B, C, P, L]
    else:
        xT_all = np.zeros((B, D, LP), dtype=np.float32)          # [B, D, L+W-1]
        xT_all[:, :, :L] = x.transpose(0, 2, 1)
        xT_all = np.ascontiguousarray(xT_all.reshape(B, C, P, LP).astype(ndt))
    cwT = np.ascontiguousarray(
        conv_w.transpose(2, 1, 0).reshape(W, C, P, D).astype(ndt)
    )                                                            # [W, C, P, o]
    pwT = np.ascontiguousarray(proj_w.T.reshape(C, P, D).astype(ndt))
    if STRUCT == "fmajor":
        pbb = np.ascontiguousarray(proj_b.reshape(C, P).T)
    else:
        pbb = np.ascontiguousarray(np.broadcast_to(proj_b[None, :], (P, D)))
    return xT_all, cwT, pwT, pbb


def kernel(x, conv_w, proj_w, proj_b):
    global LAST_RESULTS
    nc = _get_program(MODE, CUMSUM, STRUCT)
    xT_all, cwT, pwT, pbb = _prep_inputs(x, conv_w, proj_w, proj_b, MODE)
    in_maps = [
        {"xT": xT_all[b], "cw": cwT, "pw": pwT, "pb": pbb} for b in range(B)
    ]
    res = bass_utils.run_bass_kernel_spmd(
        nc, in_maps, core_ids=list(range(B)), trace=TRACE
    )
    LAST_RESULTS = res
    if STRUCT == "fmajor":
        # per-core out is [W, D, L]; final layout is [L, W, D]
        return np.stack(
            [
                np.ascontiguousarray(
                    r["out"].transpose(2, 0, 1).astype(np.float32)
                )
                for r in res.results
            ],
            axis=0,
        )
    return np.stack(
        [np.asarray(r["out"], dtype=np.float32) for r in res.results], axis=0
    )


# revision 15
# speedup vs baseline: 1.1892x; 1.1892x over previous
"""Trainium2 Bass kernel for nn_ConvShare: multi-width causal conv + shared projection.

Reference computation (per batch element b):
    xpad = pad(x[b], L -> L+W-1)                       # [L+11, D]
    taps[k]  = xpad[k:k+L, :] @ conv_w[:, :, k].T      # [L, D], k = 0..W-1
    spans[k] = cumsum_k taps                           # [L, D]
    h[k]     = relu(spans[k])
    out[:, k, :] = h[k] @ proj_w.T + proj_b            # [L, W, D]

Sharding: data-parallel over batch B=8 across the 8 NeuronCores (no
communication; conv_w/proj_w replicated per core).

On-chip layout is feature-major ([D, L], contraction dim on SBUF
partitions) for the conv stage; the proj stage streams h as the moving
operand so its output lands feature-major [o, L] and DMAs to a [W, D, L]
DRAM buffer (host transposes to [L, W, D]).

V2 structure (vs the 211us V1 baseline):
  - conv cumsum accumulates directly in 6 persistent PSUM banks
    (CUMSUM="psum"): no vector-engine adds at all.
  - proj bias+evacuation on the scalar engine (Identity activation with
    per-partition bias), so the vector engine is idle: scalar does
    6 relu + 6 evac per tap (~5us) under the PE's 15us/tap.
  - fp16 output (halves the output DMA; host upcasts to fp32).
  - WARMUP dummy matmuls with no data deps keep the PE busy from the
    startup barrier so the HAM clock gate (1.2 -> 2.4 GHz after ~3.4us
    of sustained busy) opens before the first real matmul instead of
    4us into the real stream.
  - conv tap k only streams N=512-k columns (the zero-pad tail would
    add zero; PSUM keeps the previous cumsum there), so xT needs no pad.
  - cw prefetch depth 2 and pw loaded after cw[1]: the early-tap DMA
    supply is marginal at warm clock, this removes the tap-1/2 stalls.

MODE selects the matmul input dtype (PSUM accumulation is fp32):
  "f16" (default) / "bf16" / "f32r".
"""

import os
import sys

import numpy as np

if True:  # make concourse importable regardless of harness cwd
    for _p in ("/opt/trn_rl_repo", "/opt/pypackages"):
        if _p not in sys.path and os.path.isdir(_p):
            sys.path.append(_p)

from contextlib import ExitStack  # noqa: E402

import ml_dtypes  # noqa: E402

import concourse.bacc as bacc  # noqa: E402
import concourse.bass as bass  # noqa: E402
import concourse.mybir as mybir  # noqa: E402
import concourse.tile as tile  # noqa: E402
from concourse import bass_utils  # noqa: E402

B, L, D, W = 8, 512, 768, 12
P = 128          # SBUF partitions
C = D // P       # 6 contraction chunks of 128
LP = L + W - 1   # 523: right-padded sequence length (legacy layouts only)
NB = L // P      # 4 output row blocks for proj

F32 = mybir.dt.float32
F16 = mybir.dt.float16
RELU = mybir.ActivationFunctionType.Relu
IDENT = mybir.ActivationFunctionType.Identity

MODE = "f16"     # "f32r" | "bf16" | "f16"
CUMSUM = "psum"  # "sbuf" | "psum"
STRUCT = "fmajor"  # "fmajor" ([o2,l] out, host transpose) | "lmajor" ([l,o2], direct DMA)
OUT16 = True     # fp16 DRAM output, host upcast (halves output DMA)
WARMUP = 0       # dummy matmuls: measured net-zero-to-negative. Input data
                 # lands at ~10-11.6us (DMA ring spin-up) and taps 0-1 are
                 # DMA-supply bound (~245 GB/s), so the cold 1.2 GHz phase
                 # hides under the data wait either way.
TRIM = True      # conv tap k streams only N=512-k columns
PREFETCH = 2     # cw tap prefetch depth
DELAY_PROJ = True  # issue proj(k) after conv(k+1): +7.7us of DMA lead for cw
EVAC = "vector"  # proj psum evacuation engine: "vector" | "scalar"

# Knobs the test harness may flip before calling kernel():
TRACE = False
LAST_RESULTS = None


def _build_program(mode: str, cumsum: str, struct: str) -> bass.Bass:
    mdt = {
        "f32r": mybir.dt.float32r,
        "bf16": mybir.dt.bfloat16,
        "f16": mybir.dt.float16,
    }[mode]
    odt = F16 if OUT16 else F32

    nc = bacc.Bacc(
        "TRN2",
        target_bir_lowering=False,
        debug=False,
        num_devices=B,
    )

    # DRAM I/O. Matmul inputs are pre-chunked host-side to [C, P, n] so each
    # chunk DMA is a clean 2D copy and compute can start on chunk 0 early.
    xlen = L if TRIM else LP
    xT = nc.dram_tensor("xT", [C, P, xlen], mdt, kind="ExternalInput").ap()
    cw = nc.dram_tensor("cw", [W, C, P, D], mdt, kind="ExternalInput").ap()
    pw = nc.dram_tensor("pw", [C, P, D], mdt, kind="ExternalInput").ap()
    if struct == "fmajor":
        pb = nc.dram_tensor("pb", [P, C], F32, kind="ExternalInput").ap()
        out = nc.dram_tensor("out", [W, D, L], odt, kind="ExternalOutput").ap()
    else:
        pb = nc.dram_tensor("pb", [P, D], F32, kind="ExternalInput").ap()
        out = nc.dram_tensor("out", [L, W, D], odt, kind="ExternalOutput").ap()

    with tile.TileContext(nc) as tc, ExitStack() as ctx:
        const_pool = ctx.enter_context(tc.tile_pool(name="const", bufs=1))
        cw_pool = ctx.enter_context(tc.tile_pool(name="cw", bufs=1 + PREFETCH))
        h_pool = ctx.enter_context(tc.tile_pool(name="h", bufs=2))
        out_pool = ctx.enter_context(tc.tile_pool(name="out", bufs=4))
        if cumsum == "psum":
            psc_pool = ctx.enter_context(tc.tile_pool(name="psc", bufs=1, space="PSUM"))
            psp_pool = ctx.enter_context(tc.tile_pool(name="psp", bufs=2, space="PSUM"))
        else:
            psc_pool = ctx.enter_context(tc.tile_pool(name="psc", bufs=4, space="PSUM"))
            psp_pool = ctx.enter_context(tc.tile_pool(name="psp", bufs=4, space="PSUM"))

        if WARMUP:
            # Dummy matmuls with no DMA dependencies: they run from the
            # moment the PE clears the startup barrier (~6us) and hold it
            # busy until the first input chunks land (~9.5us), so the HAM
            # clock gate (needs ~3.4us sustained busy) opens right as the
            # real stream begins. Same dtype/shape class as the real
            # matmuls (fp32 dummies hang the HW).
            wa = const_pool.tile([P, P], mdt, name="warm_a")
            wb = const_pool.tile([P, 512], mdt, name="warm_b")
            nc.gpsimd.memset(wa[:], 0.0)
            nc.gpsimd.memset(wb[:], 0.0)
            for wi in range(WARMUP):
                wp = psp_pool.tile([P, 512], F32, tag="psp", name=f"warm_ps{wi}")
                nc.tensor.matmul(
                    wp[:], lhsT=wa[:], rhs=wb[:], start=True, stop=True
                )

        def load_cw(k):
            ts = []
            for c in range(C):
                t = cw_pool.tile([P, D], mdt, tag=f"cw{c}", name=f"cw{c}_{k}")
                nc.sync.dma_start(t[:], cw[k, c, :, :])
                ts.append(t)
            return ts

        # Interleave the startup loads so the first conv matmuls (which need
        # cw[0] chunk c + xT chunk c) can begin as soon as chunk 0 lands.
        cw_tiles = {}
        xT_t = []
        for c in range(C):
            t = cw_pool.tile([P, D], mdt, tag=f"cw{c}", name=f"cw{c}_0")
            nc.sync.dma_start(t[:], cw[0, c, :, :])
            cw_tiles.setdefault(0, []).append(t)
            xt = const_pool.tile([P, xlen], mdt, tag=f"xt{c}", name=f"xt{c}")
            nc.sync.dma_start(xt[:], xT[c, :, :])
            xT_t.append(xt)

        # cw[1] interleaved chunk-wise with pw: the early phase is
        # DMA-supply bound (~245 GB/s), and conv(1) consumes cw[1] chunks
        # progressively while proj(0) starts right after conv(1) —
        # interleaving satisfies both demand curves with ~no stall
        # (cw1-then-pw stalls proj(0) by ~1us; pw-then-cw1 stalls conv(1)
        # by much more). pb LAST: tiny-row DMAs take the serial
        # DMA_DIRECT2D path on Sync (~650ns each) and must not delay the
        # descriptor writes of the bulk loads.
        pw_t = []
        for c in range(C):
            if 1 < min(PREFETCH, W):
                t = cw_pool.tile([P, D], mdt, tag=f"cw{c}", name=f"cw{c}_1")
                nc.sync.dma_start(t[:], cw[1, c, :, :])
                cw_tiles.setdefault(1, []).append(t)
            t = const_pool.tile([P, D], mdt, tag=f"pw{c}", name=f"pw{c}")
            nc.sync.dma_start(t[:], pw[c, :, :])
            pw_t.append(t)
        for k in range(2, min(PREFETCH, W)):
            cw_tiles[k] = load_cw(k)

        if struct == "fmajor":
            # single [P, C] tile: pb_tile[p, o2b] = proj_b[o2b*P + p]
            pb_tile = const_pool.tile([P, C], F32, name="pb")
            nc.sync.dma_start(pb_tile[:], pb[:, :])
            pb_t = [pb_tile[:, o2b : o2b + 1] for o2b in range(C)]
        else:
            pb_t = const_pool.tile([P, D], F32)
            nc.sync.dma_start(pb_t[:], pb[:])

        if cumsum == "psum":
            # 6 persistent PSUM banks accumulate the conv cumsum across taps.
            sp_acc = [
                psc_pool.tile([P, L], F32, tag=f"sp{ob}", name=f"sp{ob}")
                for ob in range(C)
            ]
            spans = None
        else:
            spans = const_pool.tile([P, C * L], F32)      # running conv cumsum
            nc.gpsimd.memset(spans[:], 0.0)

        def conv_stage(k, cw_cur):
            # --- conv tap k: psum[o_blk, l] += sum_d cw^T[d, o] * x^T[d, l+k]
            nk = L - k if TRIM else L
            h_t = [h_pool.tile([P, L], mdt, tag=f"h{c}", name=f"h{c}_{k}") for c in range(C)]
            for ob in range(C):
                if cumsum == "psum":
                    ps = sp_acc[ob]
                    for c in range(C):
                        nc.tensor.matmul(
                            ps[:, 0:nk],
                            lhsT=cw_cur[c][:, ob * P : (ob + 1) * P],
                            rhs=xT_t[c][:, k : k + nk],
                            start=(k == 0 and c == 0),
                            stop=(k == W - 1 and c == C - 1),
                            skip_group_check=True,
                        )
                    nc.scalar.activation(h_t[ob][:], ps[:], RELU)
                else:
                    ps = psc_pool.tile([P, L], F32, tag="psc")
                    for c in range(C):
                        nc.tensor.matmul(
                            ps[:, 0:nk],
                            lhsT=cw_cur[c][:, ob * P : (ob + 1) * P],
                            rhs=xT_t[c][:, k : k + nk],
                            start=(c == 0),
                            stop=(c == C - 1),
                        )
                    sp = spans[:, ob * L : (ob + 1) * L]
                    nc.vector.tensor_add(sp[0:P, 0:nk], sp[0:P, 0:nk], ps[:, 0:nk])
                    nc.scalar.activation(h_t[ob][:], sp, RELU)
            return h_t

        def proj_stage(k, h_t):
            if struct == "fmajor":
                # --- proj tap k (feature-major): out^T[o2_blk, l] =
                #     sum_d pw^T[d, o2] * h^T[d, l]; 36 N=512 matmuls.
                for o2b in range(C):
                    o_t = out_pool.tile([P, L], odt, tag="out", name=f"o_{k}_{o2b}")
                    pp = psp_pool.tile([P, 512], F32, tag="psp", name=f"pp_{k}_{o2b}")
                    for c in range(C):
                        nc.tensor.matmul(
                            pp[:],
                            lhsT=pw_t[c][:, o2b * P : (o2b + 1) * P],
                            rhs=h_t[c][:],
                            start=(c == 0),
                            stop=(c == C - 1),
                        )
                    # NOTE: do NOT split this into smaller column chunks —
                    # rows below ~512B flip the DMA onto the serial
                    # DMA_DIRECT2D path on the Sync engine (~650ns each),
                    # which is slower than one hardware-queue DMA.
                    if EVAC == "scalar":
                        nc.scalar.activation(o_t[:], pp[:], IDENT, bias=pb_t[o2b])
                    else:
                        nc.vector.tensor_scalar_add(o_t[:], pp[:], pb_t[o2b])
                    nc.sync.dma_start(out[k, o2b * P : (o2b + 1) * P, :], o_t[:])
            else:
                # --- proj tap k: out[l_blk, o2] = sum_d h^T[d, l]*pw^T[d, o2]+b
                for lb in range(NB):
                    o_t = out_pool.tile([P, D], odt, tag="out")
                    for n0, nn in ((0, 512), (512, 256)):
                        pp = psp_pool.tile([P, 512], F32, tag="psp")
                        for c in range(C):
                            nc.tensor.matmul(
                                pp[:, 0:nn],
                                lhsT=h_t[c][:, lb * P : (lb + 1) * P],
                                rhs=pw_t[c][:, n0 : n0 + nn],
                                start=(c == 0),
                                stop=(c == C - 1),
                            )
                        nc.vector.tensor_add(
                            o_t[:, n0 : n0 + nn], pp[:, 0:nn], pb_t[:, n0 : n0 + nn]
                        )
                    nc.sync.dma_start(out[lb * P : (lb + 1) * P, k, :], o_t[:])

        # Pipeline. With DELAY_PROJ the PE order is conv(0), conv(1),
        # proj(0), conv(2), proj(1), ..., conv(11), proj(10), proj(11):
        # each tap's cw DMA gets a full extra proj stage (~7.7us) of lead
        # time, which removes the tap-1/2 input stalls at warm clock.
        h_prev = None
        for k in range(W):
            if k + PREFETCH < W:
                cw_tiles[k + PREFETCH] = load_cw(k + PREFETCH)
            h_t = conv_stage(k, cw_tiles.pop(k))
            if not DELAY_PROJ:
                proj_stage(k, h_t)
            else:
                if h_prev is not None:
                    proj_stage(k - 1, h_prev)
                h_prev = h_t
        if DELAY_PROJ:
            proj_stage(W - 1, h_prev)

    nc.compile()
    return nc


_program_cache: dict = {}


def _get_program(mode: str, cumsum: str = None, struct: str = None) -> bass.Bass:
    if cumsum is None:
        cumsum = CUMSUM
    if struct is None:
        struct = STRUCT
    key = (mode, cumsum, struct, WARMUP, OUT16, TRIM, PREFETCH, DELAY_PROJ, EVAC)
    if key not in _program_cache:
        _program_cache[key] = _build_program(mode, cumsum, struct)
    return _program_cache[key]


def _np_dt(mode: str):
    return {"f32r": np.float32, "bf16": ml_dtypes.bfloat16, "f16": np.float16}[mode]


def _prep_inputs(x, conv_w, proj_w, proj_b, mode: str):
    x = np.asarray(x, dtype=np.float32)
    conv_w = np.asarray(conv_w, dtype=np.float32)
    proj_w = np.asarray(proj_w, dtype=np.float32)
    proj_b = np.asarray(proj_b, dtype=np.float32)
    ndt = _np_dt(mode)

    if TRIM:
        xT_all = np.ascontiguousarray(
            x.transpose(0, 2, 1).reshape(B, C, P, L).astype(ndt)
        )                                                        # [B, C, P, L]
    else:
        xT_all = np.zeros((B, D, LP), dtype=np.float32)          # [B, D, L+W-1]
        xT_all[:, :, :L] = x.transpose(0, 2, 1)
        xT_all = np.ascontiguousarray(xT_all.reshape(B, C, P, LP).astype(ndt))
    cwT = np.ascontiguousarray(
        conv_w.transpose(2, 1, 0).reshape(W, C, P, D).astype(ndt)
    )                                                            # [W, C, P, o]
    pwT = np.ascontiguousarray(proj_w.T.reshape(C, P, D).astype(ndt))
    if STRUCT == "fmajor":
        pbb = np.ascontiguousarray(proj_b.reshape(C, P).T)
    else:
        pbb = np.ascontiguousarray(np.broadcast_to(proj_b[None, :], (P, D)))
    return xT_all, cwT, pwT, pbb


def kernel(x, conv_w, proj_w, proj_b):
    global LAST_RESULTS
    nc = _get_program(MODE, CUMSUM, STRUCT)
    xT_all, cwT, pwT, pbb = _prep_inputs(x, conv_w, proj_w, proj_b, MODE)
    in_maps = [
        {"xT": xT_all[b], "cw": cwT, "pw": pwT, "pb": pbb} for b in range(B)
    ]
    res = bass_utils.run_bass_kernel_spmd(
        nc, in_maps, core_ids=list(range(B)), trace=TRACE
    )
    LAST_RESULTS = res
    if STRUCT == "fmajor":
        # per-core out is [W, D, L]; final layout is [L, W, D]
        return np.stack(
            [
                np.ascontiguousarray(
                    r["out"].transpose(2, 0, 1).astype(np.float32)
                )
                for r in res.results
            ],
            axis=0,
        )
    return np.stack(
        [np.asarray(r["out"], dtype=np.float32) for r in res.results], axis=0
    )


# revision 16
# speedup vs baseline: 1.1904x; 1.0010x over previous
"""Trainium2 Bass kernel for nn_ConvShare: multi-width causal conv + shared projection.

Reference computation (per batch element b):
    xpad = pad(x[b], L -> L+W-1)                       # [L+11, D]
    taps[k]  = xpad[k:k+L, :] @ conv_w[:, :, k].T      # [L, D], k = 0..W-1
    spans[k] = cumsum_k taps                           # [L, D]
    h[k]     = relu(spans[k])
    out[:, k, :] = h[k] @ proj_w.T + proj_b            # [L, W, D]

Sharding: data-parallel over batch B=8 across the 8 NeuronCores (no
communication; conv_w/proj_w replicated per core).

On-chip layout is feature-major ([D, L], contraction dim on SBUF
partitions) for the conv stage; the proj stage streams h as the moving
operand so its output lands feature-major [o, L] and DMAs to a [W, D, L]
DRAM buffer (host transposes to [L, W, D]).

Structure (measured 204.7-205.0us vs the 208.8-211.4us V1 baseline; PE
stream floor for this algorithm is 185.5us = 864 fp16 matmuls at the
exact warm spacing of 214.8ns, plus ~16us of fixed framework window:
~10us DMA-ring spin-up before any data + ~8.5us teardown that resets
the full 254-semaphore file):
  - conv cumsum accumulates directly in 6 persistent PSUM banks
    (CUMSUM="psum"): the vector/scalar engines never touch the cumsum.
  - relu on the scalar engine (psum -> fp16 SBUF); proj bias+evacuation
    on the vector engine (tensor_scalar_add) so neither engine exceeds
    ~half the PE's per-tap budget.
  - fp16 output (halves the output DMA; host upcasts to fp32).
  - conv tap k only streams N=512-k columns (the zero-pad tail would
    add zero; PSUM keeps the previous cumsum there), so xT needs no pad.
  - DELAY_PROJ pipeline order conv(0), conv(1), proj(0), conv(2), ...:
    each cw[k] gets a full extra stage of DMA lead time in the
    supply-bound early phase; cw[1] is interleaved chunk-wise with pw.
  - tiny DMAs (rows < ~512B) fall onto the serial DMA_DIRECT2D path on
    the Sync engine (~650ns each) — pb is one [128, 6] tile loaded after
    the bulk weights for that reason.
  - WARMUP dummy matmuls measured net-zero (taps 0-1 are DMA-supply
    bound, so the cold 1.2 GHz HAM phase hides under the data wait) and
    WARMUP=7 reproducibly tripped a ~2.0 GHz P0 power downclock
    (~244us): keep 0.

MODE selects the matmul input dtype (PSUM accumulation is fp32):
  "f16" (default) / "bf16" / "f32r".
"""

import os
import sys

import numpy as np

if True:  # make concourse importable regardless of harness cwd
    for _p in ("/opt/trn_rl_repo", "/opt/pypackages"):
        if _p not in sys.path and os.path.isdir(_p):
            sys.path.append(_p)

from contextlib import ExitStack  # noqa: E402

import ml_dtypes  # noqa: E402

import concourse.bacc as bacc  # noqa: E402
import concourse.bass as bass  # noqa: E402
import concourse.mybir as mybir  # noqa: E402
import concourse.tile as tile  # noqa: E402
from concourse import bass_utils  # noqa: E402

B, L, D, W = 8, 512, 768, 12
P = 128          # SBUF partitions
C = D // P       # 6 contraction chunks of 128
LP = L + W - 1   # 523: right-padded sequence length (legacy layouts only)
NB = L // P      # 4 output row blocks for proj

F32 = mybir.dt.float32
F16 = mybir.dt.float16
RELU = mybir.ActivationFunctionType.Relu
IDENT = mybir.ActivationFunctionType.Identity

MODE = "f16"     # "f32r" | "bf16" | "f16"
CUMSUM = "psum"  # "sbuf" | "psum"
STRUCT = "fmajor"  # "fmajor" ([o2,l] out, host transpose) | "lmajor" ([l,o2], direct DMA)
OUT16 = True     # fp16 DRAM output, host upcast (halves output DMA)
WARMUP = 0       # dummy matmuls: measured net-zero-to-negative. Input data
                 # lands at ~10-11.6us (DMA ring spin-up) and taps 0-1 are
                 # DMA-supply bound (~245 GB/s), so the cold 1.2 GHz phase
                 # hides under the data wait either way.
TRIM = True      # conv tap k streams only N=512-k columns
PREFETCH = 2     # cw tap prefetch depth
DELAY_PROJ = True  # issue proj(k) after conv(k+1): +7.7us of DMA lead for cw
EVAC = "vector"  # proj psum evacuation engine: "vector" | "scalar"

# Knobs the test harness may flip before calling kernel():
TRACE = False
LAST_RESULTS = None


def _build_program(mode: str, cumsum: str, struct: str) -> bass.Bass:
    mdt = {
        "f32r": mybir.dt.float32r,
        "bf16": mybir.dt.bfloat16,
        "f16": mybir.dt.float16,
    }[mode]
    odt = F16 if OUT16 else F32

    nc = bacc.Bacc(
        "TRN2",
        target_bir_lowering=False,
        debug=False,
        num_devices=B,
    )

    # DRAM I/O. Matmul inputs are pre-chunked host-side to [C, P, n] so each
    # chunk DMA is a clean 2D copy and compute can start on chunk 0 early.
    xlen = L if TRIM else LP
    xT = nc.dram_tensor("xT", [C, P, xlen], mdt, kind="ExternalInput").ap()
    cw = nc.dram_tensor("cw", [W, C, P, D], mdt, kind="ExternalInput").ap()
    pw = nc.dram_tensor("pw", [C, P, D], mdt, kind="ExternalInput").ap()
    if struct == "fmajor":
        pb = nc.dram_tensor("pb", [P, C], F32, kind="ExternalInput").ap()
        out = nc.dram_tensor("out", [W, D, L], odt, kind="ExternalOutput").ap()
    else:
        pb = nc.dram_tensor("pb", [P, D], F32, kind="ExternalInput").ap()
        out = nc.dram_tensor("out", [L, W, D], odt, kind="ExternalOutput").ap()

    with tile.TileContext(nc) as tc, ExitStack() as ctx:
        const_pool = ctx.enter_context(tc.tile_pool(name="const", bufs=1))
        cw_pool = ctx.enter_context(tc.tile_pool(name="cw", bufs=1 + PREFETCH))
        h_pool = ctx.enter_context(tc.tile_pool(name="h", bufs=2))
        out_pool = ctx.enter_context(tc.tile_pool(name="out", bufs=4))
        if cumsum == "psum":
            psc_pool = ctx.enter_context(tc.tile_pool(name="psc", bufs=1, space="PSUM"))
            psp_pool = ctx.enter_context(tc.tile_pool(name="psp", bufs=2, space="PSUM"))
        else:
            psc_pool = ctx.enter_context(tc.tile_pool(name="psc", bufs=4, space="PSUM"))
            psp_pool = ctx.enter_context(tc.tile_pool(name="psp", bufs=4, space="PSUM"))

        if WARMUP:
            # Dummy matmuls with no DMA dependencies: they run from the
            # moment the PE clears the startup barrier (~6us) and hold it
            # busy until the first input chunks land (~9.5us), so the HAM
            # clock gate (needs ~3.4us sustained busy) opens right as the
            # real stream begins. Same dtype/shape class as the real
            # matmuls (fp32 dummies hang the HW).
            wa = const_pool.tile([P, P], mdt, name="warm_a")
            wb = const_pool.tile([P, 512], mdt, name="warm_b")
            nc.gpsimd.memset(wa[:], 0.0)
            nc.gpsimd.memset(wb[:], 0.0)
            for wi in range(WARMUP):
                wp = psp_pool.tile([P, 512], F32, tag="psp", name=f"warm_ps{wi}")
                nc.tensor.matmul(
                    wp[:], lhsT=wa[:], rhs=wb[:], start=True, stop=True
                )

        def load_cw(k):
            ts = []
            for c in range(C):
                t = cw_pool.tile([P, D], mdt, tag=f"cw{c}", name=f"cw{c}_{k}")
                nc.sync.dma_start(t[:], cw[k, c, :, :])
                ts.append(t)
            return ts

        # Interleave the startup loads so the first conv matmuls (which need
        # cw[0] chunk c + xT chunk c) can begin as soon as chunk 0 lands.
        cw_tiles = {}
        xT_t = []
        for c in range(C):
            t = cw_pool.tile([P, D], mdt, tag=f"cw{c}", name=f"cw{c}_0")
            nc.sync.dma_start(t[:], cw[0, c, :, :])
            cw_tiles.setdefault(0, []).append(t)
            xt = const_pool.tile([P, xlen], mdt, tag=f"xt{c}", name=f"xt{c}")
            nc.sync.dma_start(xt[:], xT[c, :, :])
            xT_t.append(xt)

        # cw[1] interleaved chunk-wise with pw: the early phase is
        # DMA-supply bound (~245 GB/s), and conv(1) consumes cw[1] chunks
        # progressively while proj(0) starts right after conv(1) —
        # interleaving satisfies both demand curves with ~no stall
        # (cw1-then-pw stalls proj(0) by ~1us; pw-then-cw1 stalls conv(1)
        # by much more). pb LAST: tiny-row DMAs take the serial
        # DMA_DIRECT2D path on Sync (~650ns each) and must not delay the
        # descriptor writes of the bulk loads.
        pw_t = []
        for c in range(C):
            if 1 < min(PREFETCH, W):
                t = cw_pool.tile([P, D], mdt, tag=f"cw{c}", name=f"cw{c}_1")
                nc.sync.dma_start(t[:], cw[1, c, :, :])
                cw_tiles.setdefault(1, []).append(t)
            t = const_pool.tile([P, D], mdt, tag=f"pw{c}", name=f"pw{c}")
            nc.sync.dma_start(t[:], pw[c, :, :])
            pw_t.append(t)
        for k in range(2, min(PREFETCH, W)):
            cw_tiles[k] = load_cw(k)

        if struct == "fmajor":
            # single [P, C] tile: pb_tile[p, o2b] = proj_b[o2b*P + p]
            pb_tile = const_pool.tile([P, C], F32, name="pb")
            nc.sync.dma_start(pb_tile[:], pb[:, :])
            pb_t = [pb_tile[:, o2b : o2b + 1] for o2b in range(C)]
        else:
            pb_t = const_pool.tile([P, D], F32)
            nc.sync.dma_start(pb_t[:], pb[:])

        if cumsum == "psum":
            # 6 persistent PSUM banks accumulate the conv cumsum across taps.
            sp_acc = [
                psc_pool.tile([P, L], F32, tag=f"sp{ob}", name=f"sp{ob}")
                for ob in range(C)
            ]
            spans = None
        else:
            spans = const_pool.tile([P, C * L], F32)      # running conv cumsum
            nc.gpsimd.memset(spans[:], 0.0)

        def conv_stage(k, cw_cur):
            # --- conv tap k: psum[o_blk, l] += sum_d cw^T[d, o] * x^T[d, l+k]
            nk = L - k if TRIM else L
            h_t = [h_pool.tile([P, L], mdt, tag=f"h{c}", name=f"h{c}_{k}") for c in range(C)]
            for ob in range(C):
                if cumsum == "psum":
                    ps = sp_acc[ob]
                    for c in range(C):
                        nc.tensor.matmul(
                            ps[:, 0:nk],
                            lhsT=cw_cur[c][:, ob * P : (ob + 1) * P],
                            rhs=xT_t[c][:, k : k + nk],
                            start=(k == 0 and c == 0),
                            stop=(k == W - 1 and c == C - 1),
                            skip_group_check=True,
                        )
                    nc.scalar.activation(h_t[ob][:], ps[:], RELU)
                else:
                    ps = psc_pool.tile([P, L], F32, tag="psc")
                    for c in range(C):
                        nc.tensor.matmul(
                            ps[:, 0:nk],
                            lhsT=cw_cur[c][:, ob * P : (ob + 1) * P],
                            rhs=xT_t[c][:, k : k + nk],
                            start=(c == 0),
                            stop=(c == C - 1),
                        )
                    sp = spans[:, ob * L : (ob + 1) * L]
                    nc.vector.tensor_add(sp[0:P, 0:nk], sp[0:P, 0:nk], ps[:, 0:nk])
                    nc.scalar.activation(h_t[ob][:], sp, RELU)
            return h_t

        def proj_stage(k, h_t):
            if struct == "fmajor":
                # --- proj tap k (feature-major): out^T[o2_blk, l] =
                #     sum_d pw^T[d, o2] * h^T[d, l]; 36 N=512 matmuls.
                for o2b in range(C):
                    o_t = out_pool.tile([P, L], odt, tag="out", name=f"o_{k}_{o2b}")
                    pp = psp_pool.tile([P, 512], F32, tag="psp", name=f"pp_{k}_{o2b}")
                    for c in range(C):
                        nc.tensor.matmul(
                            pp[:],
                            lhsT=pw_t[c][:, o2b * P : (o2b + 1) * P],
                            rhs=h_t[c][:],
                            start=(c == 0),
                            stop=(c == C - 1),
                        )
                    # NOTE: do NOT split this into smaller column chunks —
                    # rows below ~512B flip the DMA onto the serial
                    # DMA_DIRECT2D path on the Sync engine (~650ns each),
                    # which is slower than one hardware-queue DMA.
                    if EVAC == "scalar":
                        nc.scalar.activation(o_t[:], pp[:], IDENT, bias=pb_t[o2b])
                    else:
                        nc.vector.tensor_scalar_add(o_t[:], pp[:], pb_t[o2b])
                    nc.sync.dma_start(out[k, o2b * P : (o2b + 1) * P, :], o_t[:])
            else:
                # --- proj tap k: out[l_blk, o2] = sum_d h^T[d, l]*pw^T[d, o2]+b
                for lb in range(NB):
                    o_t = out_pool.tile([P, D], odt, tag="out")
                    for n0, nn in ((0, 512), (512, 256)):
                        pp = psp_pool.tile([P, 512], F32, tag="psp")
                        for c in range(C):
                            nc.tensor.matmul(
                                pp[:, 0:nn],
                                lhsT=h_t[c][:, lb * P : (lb + 1) * P],
                                rhs=pw_t[c][:, n0 : n0 + nn],
                                start=(c == 0),
                                stop=(c == C - 1),
                            )
                        nc.vector.tensor_add(
                            o_t[:, n0 : n0 + nn], pp[:, 0:nn], pb_t[:, n0 : n0 + nn]
                        )
                    nc.sync.dma_start(out[lb * P : (lb + 1) * P, k, :], o_t[:])

        # Pipeline. With DELAY_PROJ the PE order is conv(0), conv(1),
        # proj(0), conv(2), proj(1), ..., conv(11), proj(10), proj(11):
        # each tap's cw DMA gets a full extra proj stage (~7.7us) of lead
        # time, which removes the tap-1/2 input stalls at warm clock.
        h_prev = None
        for k in range(W):
            if k + PREFETCH < W:
                cw_tiles[k + PREFETCH] = load_cw(k + PREFETCH)
            h_t = conv_stage(k, cw_tiles.pop(k))
            if not DELAY_PROJ:
                proj_stage(k, h_t)
            else:
                if h_prev is not None:
                    proj_stage(k - 1, h_prev)
                h_prev = h_t
        if DELAY_PROJ:
            proj_stage(W - 1, h_prev)

    nc.compile()
    return nc


_program_cache: dict = {}


def _get_program(mode: str, cumsum: str = None, struct: str = None) -> bass.Bass:
    if cumsum is None:
        cumsum = CUMSUM
    if struct is None:
        struct = STRUCT
    key = (mode, cumsum, struct, WARMUP, OUT16, TRIM, PREFETCH, DELAY_PROJ, EVAC)
    if key not in _program_cache:
        _program_cache[key] = _build_program(mode, cumsum, struct)
    return _program_cache[key]


def _np_dt(mode: str):
    return {"f32r": np.float32, "bf16": ml_dtypes.bfloat16, "f16": np.float16}[mode]


def _prep_inputs(x, conv_w, proj_w, proj_b, mode: str):
    x = np.asarray(x, dtype=np.float32)
    conv_w = np.asarray(conv_w, dtype=np.float32)
    proj_w = np.asarray(proj_w, dtype=np.float32)
    proj_b = np.asarray(proj_b, dtype=np.float32)
    ndt = _np_dt(mode)

    if TRIM:
        xT_all = np.ascontiguousarray(
            x.transpose(0, 2, 1).reshape(B, C, P, L).astype(ndt)
        )                                                        # [B, C, P, L]
    else:
        xT_all = np.zeros((B, D, LP), dtype=np.float32)          # [B, D, L+W-1]
        xT_all[:, :, :L] = x.transpose(0, 2, 1)
        xT_all = np.ascontiguousarray(xT_all.reshape(B, C, P, LP).astype(ndt))
    cwT = np.ascontiguousarray(
        conv_w.transpose(2, 1, 0).reshape(W, C, P, D).astype(ndt)
    )                                                            # [W, C, P, o]
    pwT = np.ascontiguousarray(proj_w.T.reshape(C, P, D).astype(ndt))
    if STRUCT == "fmajor":
        pbb = np.ascontiguousarray(proj_b.reshape(C, P).T)
    else:
        pbb = np.ascontiguousarray(np.broadcast_to(proj_b[None, :], (P, D)))
    return xT_all, cwT, pwT, pbb


def kernel(x, conv_w, proj_w, proj_b):
    global LAST_RESULTS
    nc = _get_program(MODE, CUMSUM, STRUCT)
    xT_all, cwT, pwT, pbb = _prep_inputs(x, conv_w, proj_w, proj_b, MODE)
    in_maps = [
        {"xT": xT_all[b], "cw": cwT, "pw": pwT, "pb": pbb} for b in range(B)
    ]
    res = bass_utils.run_bass_kernel_spmd(
        nc, in_maps, core_ids=list(range(B)), trace=TRACE
    )
    LAST_RESULTS = res
    if STRUCT == "fmajor":
        # per-core out is [W, D, L]; final layout is [L, W, D]
        return np.stack(
            [
                np.ascontiguousarray(
                    r["out"].transpose(2, 0, 1).astype(np.float32)
                )
                for r in res.results
            ],
            axis=0,
        )
    return np.stack(
        [np.asarray(r["out"], dtype=np.float32) for r in res.results], axis=0
    )


# revision 20
# speedup vs baseline: 1.1913x; 1.0008x over previous
"""Trainium2 Bass kernel for nn_ConvShare: multi-width causal conv + shared projection.

Reference computation (per batch element b):
    xpad = pad(x[b], L -> L+W-1)                       # [L+11, D]
    taps[k]  = xpad[k:k+L, :] @ conv_w[:, :, k].T      # [L, D], k = 0..W-1
    spans[k] = cumsum_k taps                           # [L, D]
    h[k]     = relu(spans[k])
    out[:, k, :] = h[k] @ proj_w.T + proj_b            # [L, W, D]

Sharding: data-parallel over batch B=8 across the 8 NeuronCores (no
communication; conv_w/proj_w replicated per core).

On-chip layout is feature-major ([D, L], contraction dim on SBUF
partitions) for the conv stage; the proj stage streams h as the moving
operand so its output lands feature-major [o, L] and DMAs to a [W, D, L]
DRAM buffer (host transposes to [L, W, D]).

Structure (measured 204.7-205.0us vs the 208.8-211.4us V1 baseline; PE
stream floor for this algorithm is 185.5us = 864 fp16 matmuls at the
exact warm spacing of 214.8ns, plus ~16us of fixed framework window:
~10us DMA-ring spin-up before any data + ~8.5us teardown that resets
the full 254-semaphore file):
  - conv cumsum accumulates directly in 6 persistent PSUM banks
    (CUMSUM="psum"): the vector/scalar engines never touch the cumsum.
  - relu on the scalar engine (psum -> fp16 SBUF); proj bias+evacuation
    on the vector engine (tensor_scalar_add) so neither engine exceeds
    ~half the PE's per-tap budget.
  - fp16 output (halves the output DMA; host upcasts to fp32).
  - conv tap k only streams N=512-k columns (the zero-pad tail would
    add zero; PSUM keeps the previous cumsum there), so xT needs no pad.
  - DELAY_PROJ pipeline order conv(0), conv(1), proj(0), conv(2), ...:
    each cw[k] gets a full extra stage of DMA lead time in the
    supply-bound early phase; cw[1] is interleaved chunk-wise with pw.
  - tiny DMAs (rows < ~512B) fall onto the serial DMA_DIRECT2D path on
    the Sync engine (~650ns each) — pb is one [128, 6] tile loaded after
    the bulk weights for that reason.
  - WARMUP dummy matmuls measured net-zero (taps 0-1 are DMA-supply
    bound, so the cold 1.2 GHz HAM phase hides under the data wait) and
    WARMUP=7 reproducibly tripped a ~2.0 GHz P0 power downclock
    (~244us): keep 0.

MODE selects the matmul input dtype (PSUM accumulation is fp32):
  "f16" (default) / "bf16" / "f32r".
"""

import os
import sys

import numpy as np

if True:  # make concourse importable regardless of harness cwd
    for _p in ("/opt/trn_rl_repo", "/opt/pypackages"):
        if _p not in sys.path and os.path.isdir(_p):
            sys.path.append(_p)

from contextlib import ExitStack  # noqa: E402

import ml_dtypes  # noqa: E402

import concourse.bacc as bacc  # noqa: E402
import concourse.bass as bass  # noqa: E402
import concourse.mybir as mybir  # noqa: E402
import concourse.tile as tile  # noqa: E402
from concourse import bass_utils  # noqa: E402

B, L, D, W = 8, 512, 768, 12
P = 128          # SBUF partitions
C = D // P       # 6 contraction chunks of 128
LP = L + W - 1   # 523: right-padded sequence length (legacy layouts only)
NB = L // P      # 4 output row blocks for proj

F32 = mybir.dt.float32
F16 = mybir.dt.float16
RELU = mybir.ActivationFunctionType.Relu
IDENT = mybir.ActivationFunctionType.Identity

MODE = "f16"     # "f32r" | "bf16" | "f16"
CUMSUM = "psum"  # "sbuf" | "psum"
STRUCT = "fmajor"  # "fmajor" ([o2,l] out, host transpose) | "lmajor" ([l,o2], direct DMA)
OUT16 = True     # fp16 DRAM output, host upcast (halves output DMA)
WARMUP = 0       # dummy matmuls: measured net-zero-to-negative. Input data
                 # lands at ~10-11.6us (DMA ring spin-up) and taps 0-1 are
                 # DMA-supply bound (~245 GB/s), so the cold 1.2 GHz phase
                 # hides under the data wait either way.
TRIM = True      # conv tap k streams only N=512-k columns
PREFETCH = 2     # cw tap prefetch depth
DELAY_PROJ = True  # issue proj(k) after conv(k+1): +7.7us of DMA lead for cw
EVAC = "vector"  # proj psum evacuation engine: "vector" | "scalar"
TAILSPLIT = 2    # split the final evac+DMA into halves (512B rows)

# Knobs the test harness may flip before calling kernel():
TRACE = False
LAST_RESULTS = None


def _build_program(mode: str, cumsum: str, struct: str) -> bass.Bass:
    mdt = {
        "f32r": mybir.dt.float32r,
        "bf16": mybir.dt.bfloat16,
        "f16": mybir.dt.float16,
    }[mode]
    odt = F16 if OUT16 else F32

    nc = bacc.Bacc(
        "TRN2",
        target_bir_lowering=False,
        debug=False,
        num_devices=B,
    )

    # DRAM I/O. Matmul inputs are pre-chunked host-side to [C, P, n] so each
    # chunk DMA is a clean 2D copy and compute can start on chunk 0 early.
    xlen = L if TRIM else LP
    xT = nc.dram_tensor("xT", [C, P, xlen], mdt, kind="ExternalInput").ap()
    cw = nc.dram_tensor("cw", [W, C, P, D], mdt, kind="ExternalInput").ap()
    pw = nc.dram_tensor("pw", [C, P, D], mdt, kind="ExternalInput").ap()
    if struct == "fmajor":
        pb = nc.dram_tensor("pb", [P, C], F32, kind="ExternalInput").ap()
        out = nc.dram_tensor("out", [W, D, L], odt, kind="ExternalOutput").ap()
    else:
        pb = nc.dram_tensor("pb", [P, D], F32, kind="ExternalInput").ap()
        out = nc.dram_tensor("out", [L, W, D], odt, kind="ExternalOutput").ap()

    with tile.TileContext(nc) as tc, ExitStack() as ctx:
        const_pool = ctx.enter_context(tc.tile_pool(name="const", bufs=1))
        cw_pool = ctx.enter_context(tc.tile_pool(name="cw", bufs=1 + PREFETCH))
        h_pool = ctx.enter_context(tc.tile_pool(name="h", bufs=2))
        out_pool = ctx.enter_context(tc.tile_pool(name="out", bufs=4))
        if cumsum == "psum":
            psc_pool = ctx.enter_context(tc.tile_pool(name="psc", bufs=1, space="PSUM"))
            psp_pool = ctx.enter_context(tc.tile_pool(name="psp", bufs=2, space="PSUM"))
        else:
            psc_pool = ctx.enter_context(tc.tile_pool(name="psc", bufs=4, space="PSUM"))
            psp_pool = ctx.enter_context(tc.tile_pool(name="psp", bufs=4, space="PSUM"))

        if WARMUP:
            # Dummy matmuls with no DMA dependencies: they run from the
            # moment the PE clears the startup barrier (~6us) and hold it
            # busy until the first input chunks land (~9.5us), so the HAM
            # clock gate (needs ~3.4us sustained busy) opens right as the
            # real stream begins. Same dtype/shape class as the real
            # matmuls (fp32 dummies hang the HW).
            wa = const_pool.tile([P, P], mdt, name="warm_a")
            wb = const_pool.tile([P, 512], mdt, name="warm_b")
            nc.gpsimd.memset(wa[:], 0.0)
            nc.gpsimd.memset(wb[:], 0.0)
            for wi in range(WARMUP):
                wp = psp_pool.tile([P, 512], F32, tag="psp", name=f"warm_ps{wi}")
                nc.tensor.matmul(
                    wp[:], lhsT=wa[:], rhs=wb[:], start=True, stop=True
                )

        def load_cw(k):
            ts = []
            for c in range(C):
                t = cw_pool.tile([P, D], mdt, tag=f"cw{c}", name=f"cw{c}_{k}")
                nc.sync.dma_start(t[:], cw[k, c, :, :])
                ts.append(t)
            return ts

        # Interleave the startup loads so the first conv matmuls (which need
        # cw[0] chunk c + xT chunk c) can begin as soon as chunk 0 lands.
        cw_tiles = {}
        xT_t = []
        for c in range(C):
            t = cw_pool.tile([P, D], mdt, tag=f"cw{c}", name=f"cw{c}_0")
            nc.sync.dma_start(t[:], cw[0, c, :, :])
            cw_tiles.setdefault(0, []).append(t)
            xt = const_pool.tile([P, xlen], mdt, tag=f"xt{c}", name=f"xt{c}")
            nc.sync.dma_start(xt[:], xT[c, :, :])
            xT_t.append(xt)

        # cw[1] interleaved chunk-wise with pw: the early phase is
        # DMA-supply bound (~245 GB/s), and conv(1) consumes cw[1] chunks
        # progressively while proj(0) starts right after conv(1) —
        # interleaving satisfies both demand curves with ~no stall
        # (cw1-then-pw stalls proj(0) by ~1us; pw-then-cw1 stalls conv(1)
        # by much more). pb LAST: tiny-row DMAs take the serial
        # DMA_DIRECT2D path on Sync (~650ns each) and must not delay the
        # descriptor writes of the bulk loads.
        # 2:1 ratio, cw1-heavy: conv(1) consumes cw1 chunk c at ~1.28us
        # intervals while proj(0) needs all pw chunks only after conv(1)
        # ends; front-loading cw1 trades proj(0) slack for conv(1) stalls.
        pw_t = [None] * C
        if 1 < min(PREFETCH, W):
            order = [("cw", 0), ("cw", 1), ("pw", 0), ("cw", 2), ("cw", 3),
                     ("pw", 1), ("cw", 4), ("cw", 5), ("pw", 2), ("pw", 3),
                     ("pw", 4), ("pw", 5)]
        else:
            order = [("pw", c) for c in range(C)]
        for kind, c in order:
            if kind == "cw":
                t = cw_pool.tile([P, D], mdt, tag=f"cw{c}", name=f"cw{c}_1")
                nc.sync.dma_start(t[:], cw[1, c, :, :])
                cw_tiles.setdefault(1, []).append(t)
            else:
                t = const_pool.tile([P, D], mdt, tag=f"pw{c}", name=f"pw{c}")
                nc.sync.dma_start(t[:], pw[c, :, :])
                pw_t[c] = t
        for k in range(2, min(PREFETCH, W)):
            cw_tiles[k] = load_cw(k)

        if struct == "fmajor":
            # single [P, C] tile: pb_tile[p, o2b] = proj_b[o2b*P + p]
            pb_tile = const_pool.tile([P, C], F32, name="pb")
            nc.sync.dma_start(pb_tile[:], pb[:, :])
            pb_t = [pb_tile[:, o2b : o2b + 1] for o2b in range(C)]
        else:
            pb_t = const_pool.tile([P, D], F32)
            nc.sync.dma_start(pb_t[:], pb[:])

        if cumsum == "psum":
            # 6 persistent PSUM banks accumulate the conv cumsum across taps.
            sp_acc = [
                psc_pool.tile([P, L], F32, tag=f"sp{ob}", name=f"sp{ob}")
                for ob in range(C)
            ]
            spans = None
        else:
            spans = const_pool.tile([P, C * L], F32)      # running conv cumsum
            nc.gpsimd.memset(spans[:], 0.0)

        def conv_stage(k, cw_cur):
            # --- conv tap k: psum[o_blk, l] += sum_d cw^T[d, o] * x^T[d, l+k]
            nk = L - k if TRIM else L
            h_t = [h_pool.tile([P, L], mdt, tag=f"h{c}", name=f"h{c}_{k}") for c in range(C)]
            for ob in range(C):
                if cumsum == "psum":
                    ps = sp_acc[ob]
                    for c in range(C):
                        nc.tensor.matmul(
                            ps[:, 0:nk],
                            lhsT=cw_cur[c][:, ob * P : (ob + 1) * P],
                            rhs=xT_t[c][:, k : k + nk],
                            start=(k == 0 and c == 0),
                            stop=(k == W - 1 and c == C - 1),
                            skip_group_check=True,
                        )
                    nc.scalar.activation(h_t[ob][:], ps[:], RELU)
                else:
                    ps = psc_pool.tile([P, L], F32, tag="psc")
                    for c in range(C):
                        nc.tensor.matmul(
                            ps[:, 0:nk],
                            lhsT=cw_cur[c][:, ob * P : (ob + 1) * P],
                            rhs=xT_t[c][:, k : k + nk],
                            start=(c == 0),
                            stop=(c == C - 1),
                        )
                    sp = spans[:, ob * L : (ob + 1) * L]
                    nc.vector.tensor_add(sp[0:P, 0:nk], sp[0:P, 0:nk], ps[:, 0:nk])
                    nc.scalar.activation(h_t[ob][:], sp, RELU)
            return h_t

        def proj_stage(k, h_t):
            if struct == "fmajor":
                # --- proj tap k (feature-major): out^T[o2_blk, l] =
                #     sum_d pw^T[d, o2] * h^T[d, l]; 36 N=512 matmuls.
                for o2b in range(C):
                    o_t = out_pool.tile([P, L], odt, tag="out", name=f"o_{k}_{o2b}")
                    pp = psp_pool.tile([P, 512], F32, tag="psp", name=f"pp_{k}_{o2b}")
                    for c in range(C):
                        nc.tensor.matmul(
                            pp[:],
                            lhsT=pw_t[c][:, o2b * P : (o2b + 1) * P],
                            rhs=h_t[c][:],
                            start=(c == 0),
                            stop=(c == C - 1),
                        )
                    # NOTE: 256B rows flip the DMA onto the serial
                    # DMA_DIRECT2D path on the Sync engine (~650ns each) —
                    # only the final group is split, and only in half
                    # (512B rows), to start its drain during the evac.
                    nsplit = TAILSPLIT if (k == W - 1 and o2b == C - 1) else 1
                    step = L // nsplit
                    for s in range(nsplit):
                        sl = slice(s * step, (s + 1) * step)
                        if EVAC == "scalar":
                            nc.scalar.activation(
                                o_t[:, sl], pp[:, sl], IDENT, bias=pb_t[o2b]
                            )
                        else:
                            nc.vector.tensor_scalar_add(
                                o_t[:, sl], pp[:, sl], pb_t[o2b]
                            )
                        nc.sync.dma_start(
                            out[k, o2b * P : (o2b + 1) * P, sl], o_t[:, sl]
                        )
            else:
                # --- proj tap k: out[l_blk, o2] = sum_d h^T[d, l]*pw^T[d, o2]+b
                for lb in range(NB):
                    o_t = out_pool.tile([P, D], odt, tag="out")
                    for n0, nn in ((0, 512), (512, 256)):
                        pp = psp_pool.tile([P, 512], F32, tag="psp")
                        for c in range(C):
                            nc.tensor.matmul(
                                pp[:, 0:nn],
                                lhsT=h_t[c][:, lb * P : (lb + 1) * P],
                                rhs=pw_t[c][:, n0 : n0 + nn],
                                start=(c == 0),
                                stop=(c == C - 1),
                            )
                        nc.vector.tensor_add(
                            o_t[:, n0 : n0 + nn], pp[:, 0:nn], pb_t[:, n0 : n0 + nn]
                        )
                    nc.sync.dma_start(out[lb * P : (lb + 1) * P, k, :], o_t[:])

        # Pipeline. With DELAY_PROJ the PE order is conv(0), conv(1),
        # proj(0), conv(2), proj(1), ..., conv(11), proj(10), proj(11):
        # each tap's cw DMA gets a full extra proj stage (~7.7us) of lead
        # time, which removes the tap-1/2 input stalls at warm clock.
        h_prev = None
        for k in range(W):
            if k + PREFETCH < W:
                cw_tiles[k + PREFETCH] = load_cw(k + PREFETCH)
            h_t = conv_stage(k, cw_tiles.pop(k))
            if not DELAY_PROJ:
                proj_stage(k, h_t)
            else:
                if h_prev is not None:
                    proj_stage(k - 1, h_prev)
                h_prev = h_t
        if DELAY_PROJ:
            proj_stage(W - 1, h_prev)

    nc.compile()
    return nc


_program_cache: dict = {}


def _get_program(mode: str, cumsum: str = None, struct: str = None) -> bass.Bass:
    if cumsum is None:
        cumsum = CUMSUM
    if struct is None:
        struct = STRUCT
    key = (mode, cumsum, struct, WARMUP, OUT16, TRIM, PREFETCH, DELAY_PROJ, EVAC, TAILSPLIT)
    if key not in _program_cache:
        _program_cache[key] = _build_program(mode, cumsum, struct)
    return _program_cache[key]


def _np_dt(mode: str):
    return {"f32r": np.float32, "bf16": ml_dtypes.bfloat16, "f16": np.float16}[mode]


def _prep_inputs(x, conv_w, proj_w, proj_b, mode: str):
    x = np.asarray(x, dtype=np.float32)
    conv_w = np.asarray(conv_w, dtype=np.float32)
    proj_w = np.asarray(proj_w, dtype=np.float32)
    proj_b = np.asarray(proj_b, dtype=np.float32)
    ndt = _np_dt(mode)

    if TRIM:
        xT_all = np.ascontiguousarray(
            x.transpose(0, 2, 1).reshape(B, C, P, L).astype(ndt)
        )                                                        # [B, C, P, L]
    else:
        xT_all = np.zeros((B, D, LP), dtype=np.float32)          # [B, D, L+W-1]
        xT_all[:, :, :L] = x.transpose(0, 2, 1)
        xT_all = np.ascontiguousarray(xT_all.reshape(B, C, P, LP).astype(ndt))
    cwT = np.ascontiguousarray(
        conv_w.transpose(2, 1, 0).reshape(W, C, P, D).astype(ndt)
    )                                                            # [W, C, P, o]
    pwT = np.ascontiguousarray(proj_w.T.reshape(C, P, D).astype(ndt))
    if STRUCT == "fmajor":
        pbb = np.ascontiguousarray(proj_b.reshape(C, P).T)
    else:
        pbb = np.ascontiguousarray(np.broadcast_to(proj_b[None, :], (P, D)))
    return xT_all, cwT, pwT, pbb


def kernel(x, conv_w, proj_w, proj_b):
    global LAST_RESULTS
    nc = _get_program(MODE, CUMSUM, STRUCT)
    xT_all, cwT, pwT, pbb = _prep_inputs(x, conv_w, proj_w, proj_b, MODE)
    in_maps = [
        {"xT": xT_all[b], "cw": cwT, "pw": pwT, "pb": pbb} for b in range(B)
    ]
    res = bass_utils.run_bass_kernel_spmd(
        nc, in_maps, core_ids=list(range(B)), trace=TRACE
    )
    LAST_RESULTS = res
    if STRUCT == "fmajor":
        # per-core out is [W, D, L]; final layout is [L, W, D]
        return np.stack(
            [
                np.ascontiguousarray(
                    r["out"].transpose(2, 0, 1).astype(np.float32)
                )
                for r in res.results
            ],
            axis=0,
        )
    return np.stack(
        [np.asarray(r["out"], dtype=np.float32) for r in res.results], axis=0
    )
